# revision 8
# baseline (speedup 1.0000x reference)
"""Trainium2 Bass kernel for nn_AttentionBlock (B=8, S=1024, E=1024, H=16, D=64).

Strategy: pure data parallelism — batch element b -> NeuronCore b (8 cores,
zero collectives). Host-side algebraic folding shrinks the device work to
4 full GEMMs + attention per core:

    RoPE is a per-batch linear map R_b on each head's 64 dims (positions are
    indexed by BATCH in this model, so R_b is constant per core). Fold it and
    the 1/sqrt(D) scale into the projection weights:
        W_Q_b = (in_proj_q @ R_b @ Wq) / 8,   W_K_b = in_proj_k @ R_b @ Wk
        W_V   = in_proj_v @ Wv,               W_O   = proj_w @ out_proj_w
    Device then computes, per core (all matmuls bf16, PSUM f32):
        qp^T = W_Q_b @ Xq^T          [E, S]   (feature-major)
        kp^T = W_K_b @ Xk^T          [E, S]
        vp   = (W_V @ Xv^T)^T        [S, E]   (token-major, 65-strided + ones col)
        per head: scores^T = kp_h^T.T @ qp_h^T -> exp -> E^T
                  o_h^T = [vp_h | 1]^T @ E^T  (ones column gives softmax denom)
                  o_h^T *= 1/denom  (broadcast via gpsimd partition_broadcast)
        ob = o^T.T @ W_O^T + query   -> LayerNorm -> out
"""

import numpy as np

B, S, E, H, D = 8, 1024, 1024, 16, 64
P = 128
NT = E // P       # 8 partition tiles
CH = 512          # moving free-dim chunk
NCH = S // CH     # 2
EPS = 1e-5

_CACHE = {}
LAST_RESULT = None


def _build_nc():
    """Build the per-core Bass graph (uncompiled)."""
    import concourse.bacc as bacc
    import concourse.tile as tile
    import concourse.mybir as mybir

    bf = mybir.dt.bfloat16
    f32 = mybir.dt.float32
    AF = mybir.ActivationFunctionType
    ALU = mybir.AluOpType
    AX = mybir.AxisListType

    nc = bacc.Bacc("TRN2", target_bir_lowering=False, debug=False, num_devices=B)

    xq = nc.dram_tensor("xq_t", [E, S], bf, kind="ExternalInput").ap()
    xk = nc.dram_tensor("xk_t", [E, S], bf, kind="ExternalInput").ap()
    xv = nc.dram_tensor("xv_t", [E, S], bf, kind="ExternalInput").ap()
    wq = nc.dram_tensor("wq", [E, E], bf, kind="ExternalInput").ap()
    wk = nc.dram_tensor("wk", [E, E], bf, kind="ExternalInput").ap()
    wv = nc.dram_tensor("wv", [E, E], bf, kind="ExternalInput").ap()
    wo = nc.dram_tensor("wo", [E, E], bf, kind="ExternalInput").ap()
    resid = nc.dram_tensor("resid", [S, E], f32, kind="ExternalInput").ap()
    gamma = nc.dram_tensor("gamma_r", [P, E], f32, kind="ExternalInput").ap()
    beta = nc.dram_tensor("beta_r", [P, E], f32, kind="ExternalInput").ap()
    out = nc.dram_tensor("out", [S, E], f32, kind="ExternalOutput").ap()

    with tile.TileContext(nc) as tc:
        with tc.tile_pool(name="xin", bufs=10) as xin, \
             tc.tile_pool(name="wgt", bufs=24) as wgt, \
             tc.tile_pool(name="proj", bufs=16) as proj, \
             tc.tile_pool(name="vpp", bufs=8) as vpp, \
             tc.tile_pool(name="etp", bufs=10) as etp, \
             tc.tile_pool(name="otp", bufs=8) as otp, \
             tc.tile_pool(name="small", bufs=3) as small, \
             tc.tile_pool(name="resp", bufs=2) as resp, \
             tc.tile_pool(name="xwork", bufs=3) as xwork, \
             tc.tile_pool(name="gbp", bufs=1) as gbp, \
             tc.tile_pool(name="stats", bufs=16) as stats, \
             tc.tile_pool(name="ps_big", bufs=4, space="PSUM") as ps_big, \
             tc.tile_pool(name="ps_pv", bufs=2, space="PSUM") as ps_pv:

            # ---- load weights + inputs ----
            wq_t, wk_t, wv_t = [], [], []
            for k in range(NT):
                t = wgt.tile([P, E], bf, tag="w")
                nc.sync.dma_start(t[:], wq[k * P:(k + 1) * P, :])
                wq_t.append(t)
            for k in range(NT):
                t = wgt.tile([P, E], bf, tag="w")
                nc.sync.dma_start(t[:], wk[k * P:(k + 1) * P, :])
                wk_t.append(t)
            for k in range(NT):
                t = wgt.tile([P, E], bf, tag="w")
                nc.sync.dma_start(t[:], wv[k * P:(k + 1) * P, :])
                wv_t.append(t)
            xq_t, xk_t, xv_t = [], [], []
            for k in range(NT):
                t = xin.tile([P, S], bf, tag="x")
                nc.sync.dma_start(t[:], xq[k * P:(k + 1) * P, :])
                xq_t.append(t)
            for k in range(NT):
                t = xin.tile([P, S], bf, tag="x")
                nc.sync.dma_start(t[:], xk[k * P:(k + 1) * P, :])
                xk_t.append(t)
            for k in range(NT):
                t = xin.tile([P, S], bf, tag="x")
                nc.sync.dma_start(t[:], xv[k * P:(k + 1) * P, :])
                xv_t.append(t)
            gamma_t = gbp.tile([P, E], f32)
            nc.sync.dma_start(gamma_t[:], gamma[:])
            beta_t = gbp.tile([P, E], f32)
            nc.sync.dma_start(beta_t[:], beta[:])

            # ---- phase 1: qp^T, kp^T (feature-major), vp (token-major) ----
            qpT, kpT = [], []
            for w_t, dst in ((wq_t, qpT), (wk_t, kpT)):
                x_t = xq_t if dst is qpT else xk_t
                for m in range(NT):
                    o_t = proj.tile([P, S], bf, tag="proj")
                    dst.append(o_t)
                    for c in range(NCH):
                        ps = ps_big.tile([P, CH], f32, tag="big")
                        for k in range(NT):
                            nc.tensor.matmul(
                                ps[:],
                                w_t[k][:, m * P:(m + 1) * P],
                                x_t[k][:, c * CH:(c + 1) * CH],
                                start=(k == 0), stop=(k == NT - 1))
                        nc.scalar.copy(o_t[:, c * CH:(c + 1) * CH], ps[:])

            # vp token-major with 65-stride head layout + ones column
            VW = H * (D + 1)   # 1040
            vp_t = []
            for tt in range(NT):
                v_t = vpp.tile([P, VW], bf, tag="vp")
                vp_t.append(v_t)
                nc.vector.memset(v_t[:, D::D + 1], 1.0)
                for c in range(NCH):
                    ps = ps_big.tile([P, CH], f32, tag="big")
                    for k in range(NT):
                        nc.tensor.matmul(
                            ps[:],
                            xv_t[k][:, tt * P:(tt + 1) * P],
                            wv_t[k][:, c * CH:(c + 1) * CH],
                            start=(k == 0), stop=(k == NT - 1))
                    for hh in range(CH // D):   # 8 heads per chunk
                        h = c * (CH // D) + hh
                        nc.vector.tensor_copy(
                            v_t[:, h * (D + 1):h * (D + 1) + D],
                            ps[:, hh * D:(hh + 1) * D])

            # ---- phase 2: attention per head ----
            oT = []
            for m in range(NT):
                oT.append(otp.tile([P, S], bf, tag="ot", name=f"oT{m}"))
            for h in range(H):
                ti, off = h // 2, (h % 2) * D
                for c in range(NCH):
                    et_t = []
                    for tt in range(NT):
                        ps = ps_big.tile([P, CH], f32, tag="big")
                        nc.tensor.matmul(
                            ps[:],
                            kpT[ti][off:off + D, tt * P:(tt + 1) * P],
                            qpT[ti][off:off + D, c * CH:(c + 1) * CH],
                            start=True, stop=True)
                        e_t = etp.tile([P, CH], bf, tag="et")
                        et_t.append(e_t)
                        nc.scalar.activation(e_t[:], ps[:], AF.Exp)
                    po = ps_pv.tile([D + 1, CH], f32, tag="pv")
                    for tt in range(NT):
                        nc.tensor.matmul(
                            po[:],
                            vp_t[tt][:, h * (D + 1):(h + 1) * (D + 1)],
                            et_t[tt][:],
                            start=(tt == 0), stop=(tt == NT - 1))
                    recip = small.tile([1, CH], f32, tag="recip")
                    nc.vector.reciprocal(recip[:], po[D:D + 1, :])
                    rep = small.tile([D, CH], f32, tag="rep")
                    nc.gpsimd.partition_broadcast(rep[:], recip[:])
                    nc.vector.tensor_tensor(
                        oT[ti][off:off + D, c * CH:(c + 1) * CH],
                        po[0:D, :], rep[:], ALU.mult)

            # ---- phase 3: out-proj + residual + LayerNorm ----
            wo_t = []
            for k in range(NT):
                t = wgt.tile([P, E], bf, tag="w")
                nc.sync.dma_start(t[:], wo[k * P:(k + 1) * P, :])
                wo_t.append(t)
            inv_e = 1.0 / E
            for m in range(NT):
                res_t = resp.tile([P, E], f32, tag="res")
                nc.sync.dma_start(res_t[:], resid[m * P:(m + 1) * P, :])
                x_t = xwork.tile([P, E], f32, tag="xw")
                for c in range(NCH):
                    ps = ps_big.tile([P, CH], f32, tag="big")
                    for k in range(NT):
                        nc.tensor.matmul(
                            ps[:],
                            oT[k][:, m * P:(m + 1) * P],
                            wo_t[k][:, c * CH:(c + 1) * CH],
                            start=(k == 0), stop=(k == NT - 1))
                    nc.vector.tensor_tensor(
                        x_t[:, c * CH:(c + 1) * CH], ps[:],
                        res_t[:, c * CH:(c + 1) * CH], ALU.add)
                # LayerNorm over free dim
                ssum = stats.tile([P, 1], f32, tag="st")
                nc.vector.tensor_reduce(ssum[:], x_t[:], AX.X, ALU.add)
                mu = stats.tile([P, 1], f32, tag="st")
                nc.vector.tensor_scalar(mu[:], ssum[:], inv_e, None, ALU.mult)
                sq = xwork.tile([P, E], f32, tag="xw")
                sqs = stats.tile([P, 1], f32, tag="st")
                nc.scalar.activation(sq[:], x_t[:], AF.Square, accum_out=sqs[:])
                ex2 = stats.tile([P, 1], f32, tag="st")
                nc.vector.tensor_scalar(ex2[:], sqs[:], inv_e, EPS, ALU.mult,
                                        ALU.add)
                musq = stats.tile([P, 1], f32, tag="st")
                nc.vector.tensor_tensor(musq[:], mu[:], mu[:], ALU.mult)
                var = stats.tile([P, 1], f32, tag="st")
                nc.vector.tensor_tensor(var[:], ex2[:], musq[:], ALU.subtract)
                sd = stats.tile([P, 1], f32, tag="st")
                nc.scalar.sqrt(sd[:], var[:])
                rstd = stats.tile([P, 1], f32, tag="st")
                nc.vector.reciprocal(rstd[:], sd[:])
                xc = xwork.tile([P, E], f32, tag="xw")
                nc.vector.tensor_scalar(xc[:], x_t[:], mu[:], None, ALU.subtract)
                y_t = xwork.tile([P, E], f32, tag="xw")
                nc.vector.scalar_tensor_tensor(
                    y_t[:], xc[:], rstd[:], gamma_t[:], ALU.mult, ALU.mult)
                nc.vector.tensor_tensor(y_t[:], y_t[:], beta_t[:], ALU.add)
                nc.sync.dma_start(out[m * P:(m + 1) * P, :], y_t[:])

    return nc


def _build_device_fn():
    """Compile the Bass graph + build a persistent PJRT runner."""
    import jax
    from concourse import bass2jax
    from jax.sharding import Mesh, PartitionSpec
    from jax.experimental.shard_map import shard_map

    nc = _build_nc()
    nc.compile()

    # ---- persistent PJRT runner (jit once, reuse across calls) ----
    bass2jax.install_neuronx_cc_hook()
    import concourse.mybir as mybir_

    partition_name = (nc.partition_id_tensor.name
                      if nc.partition_id_tensor else None)
    in_names, out_names, out_avals = [], [], []
    for alloc in nc.m.functions[0].allocations:
        if not isinstance(alloc, mybir_.MemoryLocationSet):
            continue
        name = alloc.memorylocations[0].name
        if alloc.kind == "ExternalInput":
            if name != partition_name:
                in_names.append(name)
        elif alloc.kind == "ExternalOutput":
            out_names.append(name)
            out_avals.append(jax.core.ShapedArray(
                tuple(alloc.tensor_shape), mybir_.dt.np(alloc.dtype)))
    n_params = len(in_names)
    all_in_names = in_names + out_names
    if partition_name is not None:
        all_in_names.append(partition_name)

    def _body(*args):
        operands = list(args)
        if partition_name is not None:
            operands.append(bass2jax.partition_id_tensor())
        outs = bass2jax._bass_exec_p.bind(
            *operands,
            out_avals=tuple(out_avals),
            in_names=tuple(all_in_names),
            out_names=tuple(out_names),
            lowering_input_output_aliases=(),
            sim_require_finite=True,
            sim_require_nnan=True,
            nc=nc,
        )
        return tuple(outs)

    devices = jax.devices()[:B]
    mesh = Mesh(np.asarray(devices), ("core",))
    n_outs = len(out_names)
    sharded = jax.jit(
        shard_map(_body, mesh=mesh,
                  in_specs=(PartitionSpec("core"),) * (n_params + n_outs),
                  out_specs=(PartitionSpec("core"),) * n_outs,
                  check_rep=False),
        donate_argnums=tuple(range(n_params, n_params + n_outs)),
        keep_unused=True)

    from jax.sharding import NamedSharding
    shard = NamedSharding(mesh, PartitionSpec("core"))

    def _concat(in_maps):
        per_core = [[np.asarray(m[name]) for name in in_names] for m in in_maps]
        return [np.concatenate([per_core[c][i] for c in range(B)], axis=0)
                for i in range(n_params)]

    def _zeros_dev():
        return [jax.device_put(
            np.zeros((B * a.shape[0], *a.shape[1:]), a.dtype), shard)
            for a in out_avals]

    def run(in_maps):
        out_arrs = sharded(*_concat(in_maps), *_zeros_dev())
        return [
            {name: np.asarray(out_arrs[i]).reshape(B, *out_avals[i].shape)[c]
             for i, name in enumerate(out_names)}
            for c in range(B)
        ]

    def make_timed_runner(in_maps):
        """Device-resident inputs; per-call work = dispatch + device exec."""
        dev_in = [jax.device_put(a, shard) for a in _concat(in_maps)]
        jax.block_until_ready(dev_in)

        def timed_call():
            zeros = _zeros_dev()
            jax.block_until_ready(zeros)
            import time
            t0 = time.perf_counter()
            out_arrs = sharded(*dev_in, *zeros)
            jax.block_until_ready(out_arrs)
            return time.perf_counter() - t0, out_arrs

        return timed_call

    return run, in_names, make_timed_runner


def _prep_inputs(query, key, value, Wq, Wk, Wv, in_proj_w, out_proj_w,
                 proj_w, gamma, beta):
    from ml_dtypes import bfloat16

    f32 = np.float32
    query = np.asarray(query, f32)
    key = np.asarray(key, f32)
    value = np.asarray(value, f32)
    Wq, Wk, Wv = (np.asarray(a, f32) for a in (Wq, Wk, Wv))
    in_proj_w = np.asarray(in_proj_w, f32)
    out_proj_w = np.asarray(out_proj_w, f32)
    proj_w = np.asarray(proj_w, f32)
    gamma = np.asarray(gamma, f32).reshape(-1)
    beta = np.asarray(beta, f32).reshape(-1)

    Wqa, Wka, Wva = in_proj_w[:E], in_proj_w[E:2 * E], in_proj_w[2 * E:]

    inv_freq = 1.0 / (10000.0 ** (np.arange(0, D, 2, dtype=f32) / D))
    t = np.arange(B, dtype=f32)
    freqs = t[:, None] * inv_freq[None, :]
    emb = np.concatenate([freqs, freqs], -1)          # [B, D]
    cosb, sinb = np.cos(emb).astype(f32), np.sin(emb).astype(f32)

    def rope_fold(W, b):
        """R_b @ W for blockdiag per-head RoPE rotation acting on rows."""
        Wh = W.reshape(H, D, E)
        rot = np.concatenate([-Wh[:, D // 2:, :], Wh[:, :D // 2, :]], axis=1)
        return (cosb[b][None, :, None] * Wh
                + sinb[b][None, :, None] * rot).reshape(E, E)

    scale = f32(1.0 / np.sqrt(D))
    W_V = (Wva @ Wv).astype(f32)
    W_O = (proj_w @ out_proj_w).astype(f32)
    wv_d = np.ascontiguousarray(W_V.T).astype(bfloat16)
    wo_d = np.ascontiguousarray(W_O.T).astype(bfloat16)
    gamma_r = np.ascontiguousarray(np.broadcast_to(gamma, (P, E))).astype(f32)
    beta_r = np.ascontiguousarray(np.broadcast_to(beta, (P, E))).astype(f32)

    in_maps = []
    for b in range(B):
        W_Q = (Wqa @ rope_fold(Wq, b) * scale).astype(f32)
        W_K = (Wka @ rope_fold(Wk, b)).astype(f32)
        in_maps.append({
            "xq_t": np.ascontiguousarray(query[b].T).astype(bfloat16),
            "xk_t": np.ascontiguousarray(key[b].T).astype(bfloat16),
            "xv_t": np.ascontiguousarray(value[b].T).astype(bfloat16),
            "wq": np.ascontiguousarray(W_Q.T).astype(bfloat16),
            "wk": np.ascontiguousarray(W_K.T).astype(bfloat16),
            "wv": wv_d,
            "wo": wo_d,
            "resid": np.ascontiguousarray(query[b]),
            "gamma_r": gamma_r,
            "beta_r": beta_r,
        })
    return in_maps


def kernel(query, key, value, Wq, Wk, Wv, in_proj_w, out_proj_w, proj_w,
           gamma, beta):
    global LAST_RESULT
    if "run" not in _CACHE:
        _CACHE["run"] = _build_device_fn()
    run = _CACHE["run"][0]
    in_maps = _prep_inputs(query, key, value, Wq, Wk, Wv, in_proj_w,
                           out_proj_w, proj_w, gamma, beta)
    results = run(in_maps)
    LAST_RESULT = results
    return np.stack([results[b]["out"] for b in range(B)], axis=0)
